# revision 20
# baseline (speedup 1.0000x reference)
"""HardTripletloss kernel for 8x Trainium2 NeuronCores (Bass, SPMD).

Strategy (feature-dim sharding, Gram matrix on TensorE):
  - img is [49, 1048576] fp32; row 0 = anchor, rows 1:17 positives, 17:49 negatives.
  - Split the feature dim D=1048576 into 8 contiguous shards of 131072, one per core.
  - Host pre-packs each core's shard in feature-major bf16 layout:
      xin[p, t*49 + r] = img[r, shard_base + t*128 + p]   (p<128, t<1024, r<49)
    so every 128-feature chunk t is a [128, 49] SBUF slab with features on
    partitions -- exactly the TensorE contraction layout.
  - Device: 8 HWDGE DMAs (1.6 MB each, two queues: sync + scalar) stream the
    shard into SBUF; TensorE accumulates the 49x49 Gram matrix
      G += X_t^T @ X_t   over all 1024 chunks (self-loading matmuls, one
    PSUM bank).  G[0, r] are the anchor dot products, diag(G) the squared
    norms -- no anchor broadcast and no elementwise hot loop at all.
  - ScalarE copies PSUM->SBUF once; one tiny DMA exports G per core.
  - Host sums the 8 partial Grams (fp64) and runs the cos/top-k/clamp/mean
    epilogue.

Raw Bass (no Tile framework): explicit semaphore chains.
"""

from contextlib import ExitStack

import numpy as np

N_ROWS = 49
D = 1048576
N_CORES = 8
D_SHARD = D // N_CORES  # 131072
N_CHUNKS = D_SHARD // 128  # 1024 chunks of 128 features
TILE_SIZES = [64] * 16  # tiles in 128-feature chunks
assert sum(TILE_SIZES) == N_CHUNKS and all(s % 2 == 0 for s in TILE_SIZES)
TILE_OFFS = [sum(TILE_SIZES[:i]) for i in range(len(TILE_SIZES))]
N_TILES = len(TILE_SIZES)
N_WARMUP = 40  # dummy MMs to lift the PE HAM clock gate before data arrives

MARGIN = 0.3
K_POS = 4
K_NEG = 8
EPS = 1e-8

_CACHE: dict = {}


def _build():
    import concourse.bass as bass
    from concourse import mybir

    fp8 = mybir.dt.float8e4
    f32 = mybir.dt.float32

    nc = bass.Bass("TRN2", target_bir_lowering=False, debug=False)
    xin = nc.dram_tensor(
        "xin", [128, N_CHUNKS * N_ROWS], fp8, kind="ExternalInput"
    )
    # paired-chunk Gram: 2*49=98 stationary cols -> [98, 98] PSUM; host sums
    # the two 49x49 diagonal blocks (off-diagonal blocks are unused junk)
    gram = nc.dram_tensor("gram", [2 * N_ROWS, 2 * N_ROWS], f32, kind="ExternalOutput")

    with ExitStack() as ctx:
        xb = [
            ctx.enter_context(
                nc.sbuf_tensor(f"xb{i}", [128, TILE_SIZES[i] * N_ROWS], fp8)
            )
            for i in range(N_TILES)
        ]
        gram_sb = ctx.enter_context(
            nc.sbuf_tensor("gram_sb", [2 * N_ROWS, 2 * N_ROWS], f32)
        )
        psum = ctx.enter_context(nc.psum_tensor([2 * N_ROWS, 2 * N_ROWS], f32))
        warm_ps = ctx.enter_context(nc.psum_tensor([2 * N_ROWS, 2 * N_ROWS], f32))

        load_sems = [
            ctx.enter_context(nc.semaphore(f"ld{i}")) for i in range(N_TILES)
        ]  # +16 when tile i is resident
        pe_sem = ctx.enter_context(nc.semaphore("pe_sem"))  # +1 when Gram done
        cp_sem = ctx.enter_context(nc.semaphore("cp_sem"))  # +1 when copy done
        out_sem = ctx.enter_context(nc.semaphore("out_sem"))  # +16 when exported
        block = ctx.enter_context(nc.Block(no_gpsimd_drain=True))

        xin_ap = xin.ap()

        def tile_src(t):
            o = TILE_OFFS[t] * N_ROWS
            return xin_ap[:, o : o + TILE_SIZES[t] * N_ROWS]

        @block.sync
        def _(sync):
            for t in range(0, N_TILES, 2):
                sync.dma_start(out=xb[t][:, :], in_=tile_src(t)).then_inc(
                    load_sems[t], 16
                )
            sync.wait_ge(cp_sem, 1)
            sync.dma_start(out=gram.ap(), in_=gram_sb[:, :]).then_inc(out_sem, 16)
            sync.wait_ge(out_sem, 16)

        @block.scalar
        def _(scalar):
            for t in range(1, N_TILES, 2):
                scalar.dma_start(out=xb[t][:, :], in_=tile_src(t)).then_inc(
                    load_sems[t], 16
                )

        @block.vector
        def _(vector):
            # PSUM -> SBUF copy of the finished Gram (no ACT table load on DVE)
            vector.wait_ge(pe_sem, 1)
            vector.tensor_copy(gram_sb[:, :], psum[:, :]).then_inc(cp_sem, 1)

        @block.tensor
        def _(tensor):
            # Warm-up: ~4us of dummy matmuls on (uninitialized) SBUF while the
            # first tiles stream in, so the HAM clock gate opens to 2.4 GHz
            # before the real Gram stream starts.  Results go to a scratch
            # PSUM bank and are never read.
            wap = xb[0][:, 0 : 2 * N_ROWS]
            for _ in range(N_WARMUP):
                tensor.matmul(warm_ps[:, :], wap, wap, start=True, stop=True)
            for t in range(N_TILES):
                tensor.wait_ge(load_sems[t], 16)
                pairs = TILE_SIZES[t] // 2  # 2 chunks (98 cols) per matmul
                for c in range(pairs):
                    x_ap = xb[t][:, c * 2 * N_ROWS : (c + 1) * 2 * N_ROWS]
                    mm = tensor.matmul(
                        psum[:, :],
                        x_ap,
                        x_ap,
                        start=(t == 0 and c == 0),
                        stop=(t == N_TILES - 1 and c == pairs - 1),
                    )
            mm.then_inc(pe_sem, 1)

    nc.finalize()
    return nc


def _get_nc():
    if "nc" not in _CACHE:
        _CACHE["nc"] = _build()
    return _CACHE["nc"]


def _shard_inputs(img: np.ndarray) -> list[dict]:
    import ml_dtypes

    assert img.shape == (N_ROWS, D), img.shape
    x = np.asarray(img, dtype=np.float32)
    # [r, c, t, p] -> [c, p, t, r], then flatten (t, r) per core
    xr = x.reshape(N_ROWS, N_CORES, N_CHUNKS, 128).transpose(1, 3, 2, 0)
    xr = np.ascontiguousarray(xr).astype(ml_dtypes.float8_e4m3)
    xr = xr.reshape(N_CORES, 128, N_CHUNKS * N_ROWS)
    return [{"xin": xr[c]} for c in range(N_CORES)]


def _run_spmd(img: np.ndarray, **kwargs):
    """Shard the full img, run the SPMD kernel, return BassKernelResults."""
    from concourse.bass_utils import run_bass_kernel_spmd

    nc = _get_nc()
    in_maps = _shard_inputs(img)
    return run_bass_kernel_spmd(nc, in_maps, list(range(N_CORES)), **kwargs)


def _finish(results) -> np.ndarray:
    """Sum per-core partial Grams and run the tiny triplet-loss epilogue."""
    G = np.zeros((N_ROWS, N_ROWS), np.float64)
    for c in range(N_CORES):
        g2 = results[c]["gram"].astype(np.float64)
        G += g2[:N_ROWS, :N_ROWS] + g2[N_ROWS:, N_ROWS:]

    s = G[0, 1:]  # anchor . x_r
    q = np.diag(G)  # ||x_r||^2
    na_ = max(np.sqrt(q[0]), EPS)
    nb_ = np.maximum(np.sqrt(q[1:]), EPS)
    cos = s / (na_ * nb_)
    dist = 1.0 - cos
    d_p = dist[0:16]
    d_n = dist[16:48]
    mean_p = np.sort(d_p)[-K_POS:].mean()
    top_n = np.sort(d_n)[:K_NEG]
    loss = np.mean(np.maximum(mean_p - top_n + MARGIN, 0.0))
    return np.float32(loss)


def kernel(img: np.ndarray) -> np.ndarray:
    img = np.asarray(img)
    results = _run_spmd(img).results
    return _finish(results)
